# revision 1
# baseline (speedup 1.0000x reference)
"""AttentionBlockWithSkipConnection Trainium2 kernel.

Full inputs -> full output. Data-parallel over batch B=8 across 8 cores.
Each core computes one batch: GroupNorm -> qkv 1x1conv -> full 4096x4096
attention -> proj 1x1conv -> skip add.

Layout/precision strategy (channel-major middle, fp8 DoubleRow matmuls):
  x^T [C, N] fp8e4      (64 PE transposes of the fp32 x, drains convert;
      GroupNorm stats via bn_stats on the fp8 copy -- sampling error of the
      quantization is negligible over 32K samples/group)
  GroupNorm folded into the qkv weights (fp8): qkv^T = (w*a)^T x^T + bias2
  q,k channel-major fp8 [128, 4, N]; one DoubleRow matmul per (m, qt) --
      the [128, 2, *] pair dim covers the full C=256 contraction
  V token-major DIRECTLY (no transposes): v_tm[tok,d] via DoubleRow with
      lhsT = x^T tile; its bias commutes through softmax (weights sum to 1)
      and is folded through Wp into the output bias row bp4
  logits^T[k,q] = K.T @ Q, one DoubleRow matmul per (qt, kt); two-pair
      PSUM pipeline (4 banks) keeps the PE ~2 pairs ahead of ACT
  expT = exp(logits/16 - 4) -> fp8e4, ONE ACT op per kt pair ([128,2,512]
      spanning 2 banks; the shift keeps exp <= e^3.5 inside fp8 range and
      cancels exactly in the softmax quotient). ACT is the bottleneck
      engine: the exp stream is ~76us of the ~100-180us kernel.
  denominator = ones[128,2,1] DoubleRow matmul accumulated over kt pairs
      -> colsum [1, 512] on PE (frees the DVE from 21us/qt of adds)
  o_un^T = V.T @ expT   (DoubleRow fp8, flash-style PSUM accumulation)
  recip = 1/colsum broadcast to 128 partitions by a K=1 ones matmul
  av_sb = o_un^T * recip (bf16)  -> proj TOKEN-major: out[tok,c] via
      lhsT=av_sb chunks (bf16, FWL) -- no output transposes needed
  out = proj + bp4 + x  (DVE adds, fp32 skip from resident x_tm)

PSUM budget is exactly 8 banks in phase D: logits pairs 4 + den 1 + rec 1 +
av0/av1 2; the proj reuses the den/rec banks at the tail after both are
consumed. Prologue pools are scoped and released before phase D opens.
x_tm is double-buffered so the next repetition's input DMA and transposes
overlap the previous repetition's attention tail.
"""

import numpy as np

import concourse.bacc as bacc
import concourse.mybir as mybir
import concourse.tile as tile

N_CORES = 8
B, H, W, C = 8, 64, 64, 256
N = H * W  # 4096 tokens
G = 32  # groups
GS = C // G  # 8 channels per group
EPS = 1e-5
CC = C // 128  # 2 channel chunks
QT = 512  # q tile (free dim of logits/attnv matmuls)
NQ = N // QT  # 8
NK = N // 128  # 32 k tiles
NP = NK // 2  # 16 k-tile pairs (DoubleRow)
F32 = mybir.dt.float32
BF16 = mybir.dt.bfloat16
F8 = mybir.dt.float8e4
DRM = mybir.MatmulPerfMode.DoubleRow
AF = mybir.ActivationFunctionType
EXP_SHIFT = 4.0
I32 = mybir.dt.int32
import math
SCH_A = float(np.float32(2.0 ** 23 / (16.0 * math.log(2.0))))
SCH_B = float(np.float32(127.0 * 2.0 ** 23 - 366393.0
                         - 4.0 * 2.0 ** 23 / math.log(2.0)))
# pairs whose exp runs on the DVE instead of ACT (spread mid-qt, away from
# the qt-boundary tail where the DVE is busy)
DVE_EXP_PAIRS = frozenset({5, 8, 11})


def _build(repeat=1):
    nc = bacc.Bacc(
        "TRN2",
        target_bir_lowering=False,
        debug=False,
        enable_asserts=True,
        num_devices=N_CORES,
    )
    x_d = nc.dram_tensor("x", [N, C], F32, kind="ExternalInput")
    gns_d = nc.dram_tensor("gn_scale", [C], F32, kind="ExternalInput")
    gnb_d = nc.dram_tensor("gn_bias", [C], F32, kind="ExternalInput")
    wq_d = nc.dram_tensor("w_qkv", [C, 3 * C], F32, kind="ExternalInput")
    bq_d = nc.dram_tensor("b_qkv", [3 * C], F32, kind="ExternalInput")
    wp_d = nc.dram_tensor("w_proj", [C, C], F32, kind="ExternalInput")
    bp_d = nc.dram_tensor("b_proj", [C], F32, kind="ExternalInput")
    out_d = nc.dram_tensor("out", [N, C], F32, kind="ExternalOutput")

    # group-aggregation masks: gA averages 8 consecutive partitions into one
    # group row; gB broadcasts group rows back to their 128 channels.
    gA_np = np.zeros((128, 16), np.float32)
    gB_np = np.zeros((16, 128), np.float32)
    for p in range(128):
        gA_np[p, p // GS] = 1.0 / GS
        gB_np[p // GS, p] = 1.0
    gA_d = nc.inline_tensor(gA_np, "gA")
    gB_d = nc.inline_tensor(gB_np, "gB")
    ident_d = nc.inline_tensor(np.eye(128, dtype=np.float32), "ident")

    with tile.TileContext(nc) as tc:
        for _ in range(repeat):
            _body(tc, x_d, gns_d, gnb_d, wq_d, bq_d, wp_d, bp_d, out_d,
                  gA_d, gB_d, ident_d)
    nc.compile()
    return nc


def _body(tc, x_d, gns_d, gnb_d, wq_d, bq_d, wp_d, bp_d, out_d,
          gA_d, gB_d, ident_d):
    nc = tc.nc
    x_tok = x_d.ap().rearrange("(p nt) c -> p nt c", p=128)  # [128, 32, 256]
    out_tok = out_d.ap().rearrange("(p nt) c -> p nt c", p=128)

    with (
        nc.allow_low_precision("mixed-precision attention: bf16/fp8 matmul "
                               "operands, fp32 accumulation throughout"),
        tc.tile_pool(name="consts", bufs=1) as consts,
        tc.tile_pool(name="xtm", bufs=2) as xtm_pool,
        tc.tile_pool(name="xcm", bufs=1) as xcm_pool,
        tc.tile_pool(name="qkvT", bufs=1) as qkvT_pool,
        tc.tile_pool(name="vtm", bufs=1) as vtm_pool,
    ):
        # ---- input DMAs: x first (PE transposes gate on it) ----
        ident = consts.tile([128, 128], F32)
        nc.sync.dma_start(out=ident, in_=ident_d.ap())
        x_tm = xtm_pool.tile([128, 32, C], F32, tag="x_tm")  # 32KB/partition
        dma_engs = [nc.sync, nc.scalar]
        for dchunk in range(16):
            dma_engs[dchunk % 2].dma_start(
                out=x_tm[:, dchunk * 2 : (dchunk + 1) * 2, :],
                in_=x_tok[:, dchunk * 2 : (dchunk + 1) * 2, :],
            )

        # ---- weights / small constants behind the x chunks ----
        gA = consts.tile([128, 16], F32)
        nc.sync.dma_start(out=gA, in_=gA_d.ap())
        gB = consts.tile([16, 128], F32)
        nc.scalar.dma_start(out=gB, in_=gB_d.ap())
        wq_stage = consts.tile([128, CC, 3 * C], F32)
        nc.scalar.dma_start(
            out=wq_stage, in_=wq_d.ap().rearrange("(cc p) d -> p cc d", p=128)
        )
        wp_stage = consts.tile([128, CC, C], F32)
        nc.sync.dma_start(
            out=wp_stage, in_=wp_d.ap().rearrange("(cc p) d -> p cc d", p=128)
        )
        wp_bf = consts.tile([128, CC, C], BF16)
        nc.vector.tensor_copy(out=wp_bf, in_=wp_stage)
        bq = consts.tile([128, 6], F32)
        nc.sync.dma_start(
            out=bq, in_=bq_d.ap().rearrange("(m p) -> p m", p=128)
        )
        bp_stage = consts.tile([1, C], F32)
        nc.sync.dma_start(
            out=bp_stage, in_=bp_d.ap().rearrange("(a c) -> a c", a=1)
        )
        gns = consts.tile([128, CC], F32)
        nc.scalar.dma_start(
            out=gns, in_=gns_d.ap().rearrange("(cc p) -> p cc", p=128)
        )
        gnb = consts.tile([128, CC], F32)
        nc.sync.dma_start(
            out=gnb, in_=gnb_d.ap().rearrange("(cc p) -> p cc", p=128)
        )
        ones_raw = consts.tile([128, 128], F32)
        nc.vector.memset(ones_raw, 1.0)
        # denominator DR stationary: [128, 2, 16] so the pair-dim stride is
        # 16 bytes (DoubleRow LDWEIGHTS requires step % 16 == 0); only
        # [:, :, 0:1] is used as the weights column.
        ones8_t = consts.tile([128, 2, 16], F8)
        nc.vector.tensor_copy(out=ones8_t, in_=ones_raw[:, 0:32])
        ones8 = ones8_t[:, :, 0:1]
        ones_col_bf = consts.tile([1, 128], BF16)  # K=1 broadcast stationary
        nc.vector.tensor_copy(out=ones_col_bf, in_=ones_raw[0:1, :])
        ones_col_f = consts.tile([1, 128], F32)
        nc.vector.tensor_copy(out=ones_col_f, in_=ones_raw[0:1, :])
        eps_col = consts.tile([128, 1], F32)
        nc.vector.memset(eps_col, EPS)
        nshift_col = consts.tile([128, 1], F32)
        nc.vector.memset(nshift_col, -EXP_SHIFT)

        x_cm = xcm_pool.tile([128, CC, N], F8, tag="x_cm")  # 8KB/partition
        qkvT = qkvT_pool.tile([128, 4, N], F8, tag="qkvT")  # 16KB/partition
        v_tm = vtm_pool.tile([128, NK, C], F8, tag="v_tm")  # 8KB/partition
        wq_f8 = consts.tile([128, CC, 3 * C], F8)  # folded qkv weights
        bp4 = consts.tile([128, 4, C], F32)  # b_proj broadcast 128x4 rows

        with (
            tc.tile_pool(name="pro_ps", bufs=2, space="PSUM") as pro_ps,
            tc.tile_pool(name="pro_mm", bufs=2, space="PSUM") as pro_mm,
            tc.tile_pool(name="gn_stats", bufs=2) as gn_stats,
        ):
            # ---- phase A: transpose x to channel-major bf16; bn_stats
            # interleaved so statistics finish right after the last chunk ----
            stats = gn_stats.tile([128, CC, 8, 6], F32)
            for s in range(8):
                for nt in range(4 * s, 4 * s + 4):
                    # both cc chunks transpose into one [128, 256] bank so a
                    # single batched drain amortizes the PSUM-read bubble
                    ps = pro_ps.tile([128, 256], F32, tag="trx", name="ps")
                    for cc in range(CC):
                        nc.tensor.transpose(
                            ps[:, cc * 128 : (cc + 1) * 128],
                            x_tm[:, nt, cc * 128 : (cc + 1) * 128],
                            ident,
                        )
                    dst = x_cm[:, 0:CC, nt * 128 : (nt + 1) * 128]
                    if nt % 2 == 0:
                        nc.vector.tensor_copy(out=dst, in_=ps)
                    else:
                        nc.scalar.copy(out=dst, in_=ps)
                for cc in range(CC):
                    nc.vector.bn_stats(
                        out=stats[:, cc, s, :],
                        in_=x_cm[:, cc, s * 512 : (s + 1) * 512],
                    )

            # ---- groupnorm stats -> per-channel affine (a, b); both cc
            # chunks batched through one set of ops to halve sem latency ----
            ab = gn_stats.tile([128, CC, 2], F32)  # (a, b) per channel
            mv = gn_stats.tile([128, CC, 2], F32, tag="mv")
            for cc in range(CC):
                nc.vector.bn_aggr(out=mv[:, cc, :], in_=stats[:, cc, :, :])
            mv2 = gn_stats.tile([128, CC, 2], F32, tag="mv2")
            nc.vector.tensor_copy(out=mv2[:, :, 0:1], in_=mv[:, :, 0:1])
            nc.vector.tensor_mul(out=mv2[:, :, 1:2], in0=mv[:, :, 0:1],
                                 in1=mv[:, :, 0:1])
            nc.vector.tensor_add(out=mv2[:, :, 1:2], in0=mv2[:, :, 1:2],
                                 in1=mv[:, :, 1:2])
            gp = pro_ps.tile([128, 4], F32, tag="smm", name="gp",
                             bufs=1)[0:16, :]
            nc.tensor.matmul(gp, lhsT=gA, rhs=mv2, start=True, stop=True)
            gp_sb = gn_stats.tile([16, CC, 2], F32, tag="gp_sb")
            nc.vector.tensor_copy(out=gp_sb, in_=gp)
            chs = pro_ps.tile([128, 4], F32, tag="smm", name="chs", bufs=1)
            nc.tensor.matmul(chs, lhsT=gB, rhs=gp_sb, start=True, stop=True)
            chs_sb = gn_stats.tile([128, CC, 2], F32, tag="chs_sb")
            nc.vector.tensor_copy(out=chs_sb, in_=chs)
            var = gn_stats.tile([128, CC, 1], F32, tag="var")
            msq = gn_stats.tile([128, CC, 1], F32, tag="msq")
            nc.vector.tensor_mul(out=msq, in0=chs_sb[:, :, 0:1],
                                 in1=chs_sb[:, :, 0:1])
            nc.vector.tensor_sub(out=var, in0=chs_sb[:, :, 1:2], in1=msq)
            nc.scalar.activation(out=var, in_=var, func=AF.Sqrt, bias=eps_col)
            rstd = gn_stats.tile([128, CC, 1], F32, tag="rstd")
            nc.vector.reciprocal(out=rstd, in_=var)
            nc.vector.tensor_mul(
                out=ab[:, :, 0:1], in0=rstd,
                in1=gns.rearrange("p (cc one) -> p cc one", one=1),
            )
            nc.vector.tensor_mul(out=msq, in0=chs_sb[:, :, 0:1],
                                 in1=ab[:, :, 0:1])
            nc.vector.tensor_sub(
                out=ab[:, :, 1:2], in0=gnb.rearrange("p (cc one) -> p cc one", one=1),
                in1=msq,
            )

            # ---- fold the affine into the qkv weights (fp8):
            # qkv^T = (w*a)^T x^T + (w^T b + b_qkv) ----
            for m in range(6):
                for cc in range(CC):
                    nc.scalar.mul(
                        out=wq_f8[:, cc, m * 128 : (m + 1) * 128],
                        in_=wq_stage[:, cc, m * 128 : (m + 1) * 128],
                        mul=ab[:, cc, 0:1],
                    )
            bias2 = gn_stats.tile([128, 6], F32)
            for m in range(6):
                psb = pro_ps.tile([128, 2], F32, tag="smm", name="psb", bufs=1)[:, 0:1]
                for cc in range(CC):
                    nc.tensor.matmul(
                        psb,
                        lhsT=wq_stage[:, cc, m * 128 : (m + 1) * 128],
                        rhs=ab[:, cc, 1:2],
                        start=(cc == 0),
                        stop=(cc == CC - 1),
                    )
                nc.vector.tensor_add(
                    out=bias2[:, m : m + 1], in0=psb, in1=bq[:, m : m + 1]
                )

            # ---- v token-major directly: one DoubleRow matmul per token
            # tile; v bias is folded through the projection into bp4 below ----
            for nt in range(NK):
                psv = pro_mm.tile([128, C], F32, tag="qkv", name="psv",
                                  padded_shape=[128, QT])
                nc.tensor.matmul(
                    psv,
                    lhsT=x_cm[:, 0:CC, nt * 128 : (nt + 1) * 128],
                    rhs=wq_f8[:, 0:CC, 2 * C : 3 * C],
                    start=True,
                    stop=True,
                    perf_mode=DRM,
                )
                if nt % 2 == 0:
                    nc.vector.tensor_copy(out=v_tm[:, nt, :], in_=psv)
                else:
                    nc.scalar.copy(out=v_tm[:, nt, :], in_=psv)

            # effective output bias row: bp + Wp^T @ bias_v  (v bias commutes
            # through softmax since attention weights sum to 1)
            bpv_ps = pro_mm.tile([128, C], F32, tag="bp_ps", name="bpv_ps",
                                 bufs=1)[0:1, :]
            for cc in range(CC):
                nc.tensor.matmul(
                    bpv_ps,
                    lhsT=bias2[:, 4 + cc : 5 + cc],
                    rhs=wp_stage[:, cc, :],
                    start=(cc == 0),
                    stop=(cc == CC - 1),
                )
            bpe_row = gn_stats.tile([1, C], F32, tag="bpe_row")
            nc.vector.tensor_add(out=bpe_row, in0=bpv_ps, in1=bp_stage)
            bp_ps = pro_mm.tile([128, C], F32, tag="bp_ps", name="bp_ps",
                                bufs=1)
            nc.tensor.matmul(
                bp_ps, lhsT=ones_col_f, rhs=bpe_row, start=True, stop=True
            )
            for r in range(4):
                nc.vector.tensor_copy(out=bp4[:, r, :], in_=bp_ps)

            # ---- phase B: q,k channel-major (+ bias2), one DoubleRow fp8
            # matmul per (m, qt) covering the full C=256 contraction ----
            for m in range(4):
                for qt in range(NQ):
                    ps = pro_mm.tile([128, QT], F32, tag="qkv")
                    nc.tensor.matmul(
                        ps,
                        lhsT=wq_f8[:, 0:CC, m * 128 : (m + 1) * 128],
                        rhs=x_cm[:, 0:CC, qt * QT : (qt + 1) * QT],
                        start=True,
                        stop=True,
                        perf_mode=DRM,
                    )
                    if qt % 2 == 0:
                        nc.scalar.activation(
                            out=qkvT[:, m, qt * QT : (qt + 1) * QT],
                            in_=ps,
                            func=AF.Identity,
                            bias=bias2[:, m : m + 1],
                        )
                    else:
                        nc.vector.tensor_scalar_add(
                            out=qkvT[:, m, qt * QT : (qt + 1) * QT],
                            in0=ps,
                            scalar1=bias2[:, m : m + 1],
                        )


        # ---- phase D: attention + proj + skip, per q tile ----
        with (
            tc.tile_pool(name="lgp", bufs=2, space="PSUM") as lgp,
            tc.tile_pool(name="pmisc", bufs=1, space="PSUM") as pmisc,
            tc.tile_pool(name="avp", bufs=1, space="PSUM") as avp,
            tc.tile_pool(name="expp", bufs=3) as expp,
            tc.tile_pool(name="owork", bufs=2) as owork,
        ):
            def emit_lg2(qt, pair):
                """One [128, 2, 512] logits pair (two DoubleRow matmuls, each
                contracting the full C=256 via the fp8 pair dim) feeding one
                wide ACT exp. Two rotating pair-tiles (4 banks) keep the PE
                ~2 pairs ahead of the exp stream."""
                lg2 = lgp.tile([128, 2, QT], F32, tag="lg2", name="lg2")
                for j in range(2):
                    kt = 2 * pair + j
                    nc.tensor.matmul(
                        lg2[:, j, :],
                        lhsT=qkvT[:, 2:4, kt * 128 : (kt + 1) * 128],
                        rhs=qkvT[:, 0:2, qt * QT : (qt + 1) * QT],
                        start=True,
                        stop=True,
                        perf_mode=DRM,
                    )
                return lg2

            def next_lg2(qt, pair):
                if pair < NP:
                    return emit_lg2(qt, pair)
                if qt + 1 < NQ:
                    return emit_lg2(qt + 1, pair - NP)
                return None

            lg2_cur = emit_lg2(0, 0)
            lg2_nxt = emit_lg2(0, 1)
            for qt in range(NQ):
                av_ps = [
                    avp.tile([128, QT], F32, tag=f"av{cc}", name=f"av{cc}")
                    for cc in range(CC)
                ]
                # denominator bank: matmul writes row 0; the full [128, 512]
                # shape lets the proj reuse this bank at the tail (tag "den")
                den_t = pmisc.tile([128, QT], F32, tag="den", name="den_t")
                den = den_t[0:1, :]

                for pair in range(NP):
                    expT2 = expp.tile([128, 2, QT], F8, tag="expT2",
                                      name="expT2")
                    if pair in DVE_EXP_PAIRS:
                        # Schraudolph fast exp on the DVE (ACT relief):
                        # i32(x*Af+Bf) bit-pattern ~ exp(x/16-4), +-3% rel
                        ti = expp.tile([128, 2, QT], I32, tag="ti",
                                       name="ti")
                        nc.vector.tensor_scalar(
                            out=ti, in0=lg2_cur,
                            scalar1=SCH_A, scalar2=SCH_B,
                            op0=mybir.AluOpType.mult,
                            op1=mybir.AluOpType.add)
                        nc.vector.tensor_copy(out=expT2,
                                              in_=ti.bitcast(F32))
                    else:
                        # one wide exp over both banks; the -4 shift keeps
                        # the fp8 numerator in range, cancels in softmax.
                        nc.scalar.activation(
                            out=expT2,
                            in_=lg2_cur,
                            func=AF.Exp,
                            scale=1.0 / 16.0,
                            bias=nshift_col,
                        )
                    lg2_cur = lg2_nxt
                    lg2_nxt = next_lg2(qt, pair + 2)
                    for cc in range(CC):
                        nc.tensor.matmul(
                            av_ps[cc],
                            lhsT=v_tm[:, 2 * pair : 2 * pair + 2,
                                      cc * 128 : (cc + 1) * 128],
                            rhs=expT2,
                            start=(pair == 0),
                            stop=(pair == NP - 1),
                            perf_mode=DRM,
                        )
                    nc.tensor.matmul(
                        den,
                        lhsT=ones8,
                        rhs=expT2,
                        start=(pair == 0),
                        stop=(pair == NP - 1),
                        perf_mode=DRM,
                    )

                # ---- tail: softmax denominator -> normalize -> proj ----
                recip_row = owork.tile([1, QT], BF16, tag="recip_row")
                nc.vector.reciprocal(out=recip_row, in_=den)
                rec_ps = pmisc.tile([128, QT], F32, tag="rec", name="rec")
                nc.tensor.matmul(
                    rec_ps, lhsT=ones_col_bf, rhs=recip_row,
                    start=True, stop=True,
                )
                rec_sb = owork.tile([128, QT], BF16, tag="rec_sb")
                nc.vector.tensor_copy(out=rec_sb, in_=rec_ps)
                av_sb = owork.tile([128, CC, QT], BF16, tag="av_sb")
                for cc in range(CC):
                    nc.vector.tensor_mul(
                        out=av_sb[:, cc, :], in0=av_ps[cc], in1=rec_sb
                    )

                # proj token-major into the den/rec banks (both consumed by
                # now): pjA <- rec bank, pjB <- den bank; no output transposes
                pjA = pmisc.tile([128, QT], F32, tag="rec", name="pjA")
                pjB = pmisc.tile([128, QT], F32, tag="den", name="pjB")
                for tt in range(4):
                    bank = pjA if tt < 2 else pjB
                    seg = bank[:, (tt % 2) * C : (tt % 2 + 1) * C]
                    for cc in range(CC):
                        nc.tensor.matmul(
                            seg,
                            lhsT=av_sb[:, cc, tt * 128 : (tt + 1) * 128],
                            rhs=wp_bf[:, cc, :],
                            start=(cc == 0),
                            stop=(cc == CC - 1),
                        )

                out_sb = owork.tile([128, 4, C], F32, tag="out_sb")
                for half in range(2):
                    nc.vector.tensor_add(
                        out=out_sb[:, half * 2 : (half + 1) * 2, :],
                        in0=(pjA if half == 0 else pjB),
                        in1=x_tm[:, qt * 4 + half * 2 : qt * 4 + (half + 1) * 2, :],
                    )
                nc.vector.tensor_add(out=out_sb, in0=out_sb, in1=bp4)
                nc.sync.dma_start(
                    out=out_tok[:, qt * 4 : (qt + 1) * 4, :], in_=out_sb
                )


_NC = None


def _get_nc():
    global _NC
    if _NC is None:
        _NC = _build()
    return _NC


_RUNNER = None
_ZEROS_FN = None

IN_NAMES = ["x", "gn_scale", "gn_bias", "w_qkv", "b_qkv", "w_proj", "b_proj"]


def _get_runner():
    """Cached jitted shard_map executable over the 8 cores (the equivalent of
    run_bass_kernel_spmd's axon path, but built once instead of per call)."""
    global _RUNNER
    if _RUNNER is not None:
        return _RUNNER
    import jax
    from jax.sharding import Mesh, PartitionSpec
    from jax.experimental.shard_map import shard_map
    from concourse import bass2jax

    nc = _get_nc()
    bass2jax.install_neuronx_cc_hook()

    in_names = list(IN_NAMES) + ["out"]
    if nc.partition_id_tensor is not None:
        in_names.append(nc.partition_id_tensor.name)

    def _body_fn(*args):
        operands = list(args)
        if nc.partition_id_tensor is not None:
            operands.append(bass2jax.partition_id_tensor())
        outs = bass2jax._bass_exec_p.bind(
            *operands,
            out_avals=(jax.core.ShapedArray((N, C), np.float32),),
            in_names=tuple(in_names),
            out_names=("out",),
            lowering_input_output_aliases=(),
            sim_require_finite=True,
            sim_require_nnan=True,
            nc=nc,
        )
        return tuple(outs)

    devices = jax.devices()[:N_CORES]
    mesh = Mesh(np.asarray(devices), ("core",))
    in_specs = (PartitionSpec("core"),) * (len(IN_NAMES) + 1)
    out_specs = (PartitionSpec("core"),)
    sharded = jax.jit(
        shard_map(
            _body_fn, mesh=mesh, in_specs=in_specs, out_specs=out_specs,
            check_rep=False,
        ),
        donate_argnums=(len(IN_NAMES),),
        keep_unused=True,
    )
    _RUNNER = sharded
    return _RUNNER


def kernel(x, gn_scale, gn_bias, w_qkv, b_qkv, w_proj, b_proj):
    sharded = _get_runner()
    x = np.ascontiguousarray(np.asarray(x, dtype=np.float32).reshape(B * N, C))
    shared = {
        "gn_scale": np.asarray(gn_scale, np.float32),
        "gn_bias": np.asarray(gn_bias, np.float32),
        "w_qkv": np.ascontiguousarray(np.asarray(w_qkv, np.float32)),
        "b_qkv": np.asarray(b_qkv, np.float32),
        "w_proj": np.ascontiguousarray(np.asarray(w_proj, np.float32)),
        "b_proj": np.asarray(b_proj, np.float32),
    }
    # shard_map slices axis 0 across cores: x gets its own batch; the shared
    # weights are tiled 8x so every core sees an identical copy.
    concat = [x]
    for name in IN_NAMES[1:]:
        a = shared[name]
        concat.append(np.concatenate([a] * N_CORES, axis=0))
    # donated output buffer, created on-device (saves a 32MB host->device
    # transfer through the axon tunnel every call)
    import jax
    import jax.numpy as jnp
    from jax.sharding import Mesh, NamedSharding, PartitionSpec

    global _ZEROS_FN
    if _ZEROS_FN is None:
        mesh = Mesh(np.asarray(jax.devices()[:N_CORES]), ("core",))
        sh = NamedSharding(mesh, PartitionSpec("core"))
        _ZEROS_FN = jax.jit(
            lambda: jnp.zeros((N_CORES * N, C), jnp.float32), out_shardings=sh
        )
    zeros = _ZEROS_FN()
    (out,) = sharded(*concat, zeros)
    return np.asarray(out).reshape(B, H, W, C)

